# revision 12
# baseline (speedup 1.0000x reference)
"""Trainium2 Bass kernel for nn_DiscreteStateTransition (NRI-style GNN message passing).

Reference computation (per batch b, time t):
  inputs[o]   = concat(x[b,o,t,:56], forward_probs[b,o,t,:8])          # [8, 64]
  pre_msg[e]  = concat(inputs[recv(e)], inputs[send(e)])               # [56, 128]
  h1          = relu(pre_msg @ W1 + b1)                                # [56, 512]
  msg         = relu(h1 @ W2 + b2)                                     # [56, 512]
  agg[o]      = sum over edges e with recv(e)==o of msg[e]             # [8, 512]
  out[o]      = concat(inputs[o], agg[o]) @ Wn + bn                    # [8, 64]

Sharding: data-parallel over (B=4) x (T-halves=2) -> 8 cores. Each core owns one
(b, t-half) slice: [8 objects, 256 timesteps]. Weights replicated.

Feature-major on-chip layout (features on partitions, (node,time)/(edge,time) on
the free axis), fp16 matmul operands throughout (PE rate is identical to fp32r,
but 2-byte tiles unlock the DVE 2x/4x perf modes and halve SBUF).

L1 is factored through per-node partial products instead of a per-edge matmul:
  A[o] = inputs[o] @ W1[:64]  + b1      (recv half)
  B[o] = inputs[o] @ W1[64:]            (send half)
  h1[e=(r,j)] = relu(A[r] + B[s]),  s = (r+1+j) mod 8
Edges are ordered (recv r, slot j) with the CYCLIC sender convention so that,
with B duplicated along the node axis (16 slots), the whole send-side gather is
a single overlapping access pattern [slot stride 32 in both r and j]. The
edge->node aggregation is a bb-tree over j on 2-byte packed views; the output
head consumes agg via 4 small matmuls plus the input-part matmul.
"""

import contextlib

import numpy as np

import concourse.bacc as bacc
import concourse.mybir as mybir
import concourse.tile as tile
from concourse.bass_types import AP as BassAP
from concourse.bass_utils import run_bass_kernel_spmd
from concourse.masks import make_identity

F32 = mybir.dt.float32
MM_DT = mybir.dt.float16

# Problem constants (hardcoded per the harness contract).
B, O, T = 4, 8, 512
D = 64            # node feature size (56 + 8)
E = 56            # directed edges = O*(O-1)
H = 512           # msg hidden/out size
KK = 64           # K*K output features
TC = 256          # timesteps per core
TB = 32           # timesteps per chunk
NCHUNK = TC // TB
CE = E * TB       # edge columns per chunk (1792)
NN = O * TB       # node columns per chunk (256)
CB = 448          # L2 matmul column block (= 2 recv groups x 7 x TB)
NCB = CE // CB    # 4


def build_nc(mm_dt=MM_DT, repeat=1):
    """Build the per-core Bass program (same program on all 8 cores)."""
    nc = bacc.Bacc("TRN2", target_bir_lowering=False, debug=False)

    xs = nc.dram_tensor("xs", [O, TC, 56], F32, kind="ExternalInput").ap()
    fps = nc.dram_tensor("fps", [O, TC, 8], F32, kind="ExternalInput").ap()
    w1 = nc.dram_tensor("w1", [2 * D, H], F32, kind="ExternalInput").ap()
    b1 = nc.dram_tensor("b1", [H], F32, kind="ExternalInput").ap()
    w2 = nc.dram_tensor("w2", [H, H], F32, kind="ExternalInput").ap()
    b2 = nc.dram_tensor("b2", [H], F32, kind="ExternalInput").ap()
    wn = nc.dram_tensor("wn", [D + H, KK], F32, kind="ExternalInput").ap()
    bn = nc.dram_tensor("bn", [KK], F32, kind="ExternalInput").ap()
    out = nc.dram_tensor("out", [O, TC, KK], F32, kind="ExternalOutput").ap()

    AF = mybir.ActivationFunctionType
    MD = mm_dt

    with tile.TileContext(nc) as tc:
        with (
            tc.tile_pool(name="const", bufs=1) as const,
            tc.tile_pool(name="inp", bufs=3) as inp_pool,
            tc.tile_pool(name="abp", bufs=2) as ab_pool,
            tc.tile_pool(name="h1pre", bufs=2) as h1pre_pool,
            tc.tile_pool(name="h1p", bufs=2) as h1_pool,
            tc.tile_pool(name="msgp", bufs=8) as msg_pool,
            tc.tile_pool(name="tmpp", bufs=3) as tmp_pool,
            tc.tile_pool(name="aggp", bufs=4) as agg_pool,
            tc.tile_pool(name="netp", bufs=2) as net_pool,
            tc.tile_pool(name="orm", bufs=3) as orm_pool,
            tc.tile_pool(name="abps", bufs=4, space="PSUM") as ab_ps,
            tc.tile_pool(name="l2ps", bufs=3, space="PSUM") as l2ps,
            tc.tile_pool(name="opps", bufs=1, space="PSUM") as opps,
        ):
            # ---- constants / weights ----
            ident = const.tile([128, 128], F32)
            make_identity(nc, ident)

            w1s = const.tile([128, H], F32)
            nc.gpsimd.dma_start(w1s[:], w1)
            w2s = const.tile([128, 4 * H], F32)
            for k in range(4):
                nc.gpsimd.dma_start(w2s[:, k * H:(k + 1) * H], w2[k * 128:(k + 1) * 128, :])
            wns = const.tile([128, 5 * KK], F32)
            nc.gpsimd.dma_start(wns[0:64, 4 * KK:5 * KK], wn[0:64, :])
            for k in range(4):
                nc.gpsimd.dma_start(wns[:, k * KK:(k + 1) * KK],
                                    wn[64 + k * 128:64 + (k + 1) * 128, :])
            # fp16 conversions: w1a/w1b on Act (needed first, small); the rest
            # on DVE so the Act queue reaches chunk-0's evicts quickly.
            w1a = const.tile([64, H], MD)          # W1 recv-half rows 0:64
            nc.scalar.copy(w1a[:], w1s[0:64, :])
            w1b = const.tile([64, H], MD)          # W1 send-half rows 64:128
            nc.scalar.copy(w1b[:], w1s[64:128, :])
            w2t = const.tile([128, 4 * H], MD)
            nc.vector.tensor_copy(out=w2t[:], in_=w2s[:])
            wnin = const.tile([64, KK], MD)        # Wn rows 0:64 (node-input part)
            nc.vector.tensor_copy(out=wnin[:], in_=wns[0:64, 4 * KK:5 * KK])
            wnagg = const.tile([128, 4 * KK], MD)  # Wn rows 64+128k
            nc.vector.tensor_copy(out=wnagg[:], in_=wns[:, 0:4 * KK])
            b1t = const.tile([128, 4], F32)
            nc.gpsimd.dma_start(b1t[:], b1.rearrange("(f p) -> p f", p=128))
            b2t = const.tile([128, 4], F32)
            nc.gpsimd.dma_start(b2t[:], b2.rearrange("(f p) -> p f", p=128))
            bnt = const.tile([64, 1], F32)
            nc.gpsimd.dma_start(bnt[:], bn.unsqueeze(1))

            # ---- load node features, transpose to feature-major ----
            inputsT = const.tile([64, O * TC], MD)
            rms = {}
            for th in range(2):
                for o in range(O):
                    rm = inp_pool.tile([128, 64], F32, name=f"rm{th}_{o}", tag="rm")
                    nc.sync.dma_start(rm[:, 0:56], xs[o, th * 128:(th + 1) * 128, :])
                    nc.sync.dma_start(rm[:, 56:64], fps[o, th * 128:(th + 1) * 128, :])
                    rms[(th, o)] = rm

            def make_intr(th, o):
                def unit():
                    rm = rms[(th, o)]
                    tp = opps.tile([64, 128], F32, name="tp", tag="op")
                    nc.tensor.transpose(tp[:], rm[:], ident[:])
                    cb0 = o * TC + th * 128
                    nc.scalar.copy(inputsT[:, cb0:cb0 + 128], tp[:])
                return unit

            for o in range(O):
                make_intr(0, o)()
            # t-half-1 transposes: deferred into chunk-0 stalls for the
            # single-shot build; under For_i they must stay outside the loop.
            input_tq = []
            if repeat > 1:
                for o in range(O):
                    make_intr(1, o)()
            else:
                input_tq = [make_intr(1, o) for o in range(O)]

            inT = inputsT.rearrange("p (o t) -> p o t", o=O)

            def cyc_ap(b2tile, f1):
                """Overlapping send-gather view of the duplicated B tile:
                element (r, j, t) reads node slot 1+r+j (sender (r+1+j)%8)."""
                v = b2tile[:, f1, :]
                return BassAP(v.tensor, v.offset + TB,
                              [list(v.ap[0]), [TB, O], [TB, O - 1], [1, TB]])

            loop_ctx = (tc.For_i(0, repeat, 1,
                                 hint_engines=(mybir.EngineType.PE,))
                        if repeat > 1 else contextlib.nullcontext())
            with loop_ctx:
                pend_l2 = None     # L2 unit thunks from the previous chunk
                pend_agg = None    # agg thunks from the previous chunk
                pend_heads = []    # head thunks (deferred two chunks)

                def make_head(t0, inc, aggT):
                    def head():
                        np_ps = opps.tile([64, NN], F32, name="np_ps", tag="op")
                        nc.tensor.matmul(np_ps[:], wnin[:], inc,
                                         start=True, stop=False)
                        for k in range(4):
                            nc.tensor.matmul(
                                np_ps[:],
                                wnagg[:, k * KK:(k + 1) * KK],
                                aggT[:, k, :],
                                start=False, stop=(k == 3))
                        netoutT = net_pool.tile([64, NN], F32, name="netoutT")
                        nc.scalar.activation(netoutT[:], np_ps[:], AF.Identity,
                                             bias=bnt[:])
                        for hf in range(2):
                            tp2 = opps.tile([128, 64], F32, name="tp2", tag="op")
                            nc.tensor.transpose(
                                tp2[:], netoutT[:, hf * 128:(hf + 1) * 128],
                                ident[0:64, 0:64])
                            outrm = orm_pool.tile([128, 64], F32, name="outrm")
                            nc.vector.tensor_copy(out=outrm[:], in_=tp2[:])
                            o0 = hf * 4
                            nc.sync.dma_start(
                                out[o0:o0 + 4, t0:t0 + TB, :],
                                outrm[:, :])
                    return head

                for c in range(NCHUNK):
                    t0 = c * TB
                    inc = inT[:, :, t0:t0 + TB]          # [64, 8, TB]

                    # ---- A/B node matmuls + evicts (PE first, then vector) ----
                    A_s = ab_pool.tile([128, 4, NN], MD, name="A_s", tag="ab")
                    B2 = ab_pool.tile([128, 4, 2 * NN], MD, name="B2", tag="ab")
                    ab_ev = []
                    for p in range(2):
                        psA = ab_ps.tile([128, 2, NN], F32, name="psA", tag="abps")
                        for i in range(2):
                            f1 = 2 * p + i
                            nc.tensor.matmul(
                                psA[:, i, :], w1a[:, f1 * 128:(f1 + 1) * 128],
                                inc, start=True, stop=True)

                        def ev_a(psA=psA, p=p):
                            for i in range(2):
                                f1 = 2 * p + i
                                nc.scalar.activation(
                                    A_s[:, f1, :], psA[:, i, :], AF.Identity,
                                    bias=b1t[:, f1:f1 + 1])
                        ab_ev.append(ev_a)
                    for p in range(2):
                        psB = ab_ps.tile([128, 2, NN], F32, name="psB", tag="abps")
                        for i in range(2):
                            f1 = 2 * p + i
                            nc.tensor.matmul(
                                psB[:, i, :], w1b[:, f1 * 128:(f1 + 1) * 128],
                                inc, start=True, stop=True)

                        def ev_b(psB=psB, p=p):
                            for i in range(2):
                                f1 = 2 * p + i
                                nc.vector.tensor_copy(out=B2[:, f1, 0:NN],
                                                      in_=psB[:, i, :])
                                nc.vector.tensor_copy(out=B2[:, f1, NN:2 * NN],
                                                      in_=B2[:, f1, 0:NN])
                        ab_ev.append(ev_b)
                    for ev in ab_ev:
                        ev()

                    # ---- h1 build: one wide DVE add (2x) + one 4x relu ----
                    h1 = h1_pool.tile([128, 4, CE], MD, name="h1")
                    h1p = h1pre_pool.tile([128, 4, CE], MD, name="h1p")

                    def make_h1add(fs, A_s=A_s, B2=B2, h1p=h1p):
                        def unit():
                            nf = len(fs)
                            f0 = fs[0]
                            dst = (h1p[:, f0:f0 + nf, :]
                                   .rearrange("p f (r j t) -> p f r j t",
                                              r=O, j=O - 1))
                            a_in = (A_s[:, f0:f0 + nf, :]
                                    .rearrange("p f (r t) -> p f r t", r=O)
                                    .unsqueeze(3)
                                    .broadcast_to([128, nf, O, O - 1, TB]))
                            v = B2[:, f0, :]
                            b_in = BassAP(v.tensor, v.offset + TB,
                                          [list(v.ap[0]), [2 * NN, nf],
                                           [TB, O], [TB, O - 1], [1, TB]])
                            nc.vector.tensor_add(dst, a_in, b_in)
                        return unit

                    def h1relu(h1=h1, h1p=h1p):
                        nc.vector.tensor_relu(out=h1[:], in_=h1p[:])

                    h1q = [make_h1add((0, 1)), make_h1add((2, 3)), h1relu]

                    # ---- this chunk's L2 / agg units (run next iteration) ----
                    aggT = agg_pool.tile([128, 4, NN], MD, name="aggT")
                    msgs = [msg_pool.tile([128, CE], MD, name=f"msg{f2}", tag="msg")
                            for f2 in range(4)]

                    def make_l2(f2, cb, h1=h1, msgs=msgs):
                        def unit():
                            mp = l2ps.tile([128, CB], F32, name="mp", tag="l2")
                            for k in range(4):
                                nc.tensor.matmul(
                                    mp[:],
                                    w2t[:, k * H + f2 * 128:k * H + (f2 + 1) * 128],
                                    h1[:, k, cb * CB:(cb + 1) * CB],
                                    start=(k == 0), stop=(k == 3))
                            # evict with a permuted write: psum cols are
                            # (rr, j, t); msg layout is (j, r, t) so the agg
                            # tree over j reads 2-byte-packed slices.
                            msgv = (msgs[f2][:]
                                    .rearrange("p (j r t) -> p j r t", j=O - 1, r=O)
                                    [:, :, 2 * cb:2 * cb + 2, :]
                                    .transpose([0, 2, 1, 3]))
                            nc.scalar.activation(
                                msgv,
                                mp[:].rearrange("p (r j t) -> p r j t",
                                                r=2, j=O - 1),
                                AF.Relu, bias=b2t[:, f2:f2 + 1])
                        return unit

                    def make_agg(f2, msgs=msgs, aggT=aggT):
                        def unit():
                            # DVE runs these at 2x (2-byte packed); Pool takes
                            # two trees since its adds are ~2x slower.
                            eng = nc.vector if f2 < 2 else nc.gpsimd
                            mg = msgs[f2][:].rearrange("p (j rt) -> p j rt",
                                                       j=O - 1)
                            tm1 = tmp_pool.tile([128, 3, NN], MD, name="tm1")
                            eng.tensor_add(tm1[:], mg[:, 0:3, :],
                                           mg[:, 3:6, :])
                            tm2 = tmp_pool.tile([128, NN], MD, name="tm2")
                            eng.tensor_add(tm2[:], tm1[:, 0, :],
                                           tm1[:, 1, :])
                            tm3 = tmp_pool.tile([128, NN], MD, name="tm3")
                            eng.tensor_add(tm3[:], tm2[:], tm1[:, 2, :])
                            eng.tensor_add(aggT[:, f2, :], tm3[:],
                                           mg[:, 6, :])
                        return unit

                    # ---- emit: head(c-3), then L2(c-1) interleaved with
                    # h1(c) and agg(c-1) ----
                    if len(pend_heads) >= 3:
                        pend_heads.pop(0)()
                    l2q = list(pend_l2) if pend_l2 else []
                    aggq = list(pend_agg) if pend_agg else []
                    if l2q:
                        for i, u in enumerate(l2q):
                            u()
                            if i % 4 == 1 and h1q:
                                h1q.pop(0)()
                            elif i % 4 == 3 and aggq:
                                aggq.pop(0)()
                            elif input_tq:
                                input_tq.pop(0)()
                    for u in h1q:
                        u()
                        if input_tq:
                            input_tq.pop(0)()
                    for u in aggq:
                        u()

                    # f2 order (2,3,0,1): the Pool-assigned agg trees (f2 2,3)
                    # get their msgs first, hiding Pool's slow adds.
                    F2ORD = (2, 3, 0, 1)
                    pend_l2 = [make_l2(f2, cb) for f2 in F2ORD
                               for cb in range(NCB)]
                    pend_agg = [make_agg(f2) for f2 in F2ORD]
                    pend_heads.append(make_head(t0, inc, aggT))

                # ---- drain the software pipeline ----
                while len(pend_heads) >= 3:
                    pend_heads.pop(0)()
                for i, u in enumerate(pend_l2):
                    u()
                    if i % 4 == 3 and pend_agg:
                        pend_agg.pop(0)()
                for u in pend_agg or []:
                    u()
                for hthunk in pend_heads:
                    hthunk()

    nc.compile()
    return nc


_NC_CACHE = {}


def _get_nc():
    key = (MM_DT, 1)
    if key not in _NC_CACHE:
        _NC_CACHE[key] = build_nc(MM_DT, 1)
    return _NC_CACHE[key]


def shard_inputs(x, forward_probs, **_):
    x = np.ascontiguousarray(np.asarray(x, dtype=np.float32))
    fp = np.ascontiguousarray(np.asarray(forward_probs, dtype=np.float32))
    in_maps = []
    for c in range(8):
        b, th = c // 2, c % 2
        in_maps.append({
            "xs": np.ascontiguousarray(x[b, :, th * TC:(th + 1) * TC, :]),
            "fps": np.ascontiguousarray(fp[b, :, th * TC:(th + 1) * TC, :]),
        })
    return in_maps


def kernel(y, x, hidden_states, forward_probs, edge_est, edge_gt,
           W1, b1, W2, b2, Wn, bn, edge2node):
    nc = _get_nc()
    weights = {
        "w1": np.ascontiguousarray(np.asarray(W1, dtype=np.float32)),
        "b1": np.ascontiguousarray(np.asarray(b1, dtype=np.float32)),
        "w2": np.ascontiguousarray(np.asarray(W2, dtype=np.float32)),
        "b2": np.ascontiguousarray(np.asarray(b2, dtype=np.float32)),
        "wn": np.ascontiguousarray(np.asarray(Wn, dtype=np.float32)),
        "bn": np.ascontiguousarray(np.asarray(bn, dtype=np.float32)),
    }
    in_maps = [dict(m, **weights) for m in shard_inputs(x, forward_probs)]
    res = run_bass_kernel_spmd(nc, in_maps, list(range(8)))
    full = np.empty((B, O, T, KK), dtype=np.float32)
    for c in range(8):
        b, th = c // 2, c % 2
        full[b, :, th * TC:(th + 1) * TC, :] = res.results[c]["out"]
    return full.reshape(B, O, T, 8, 8)


# revision 16
# speedup vs baseline: 1.3439x; 1.3439x over previous
"""Trainium2 Bass kernel for nn_DiscreteStateTransition (NRI-style GNN message passing).

Reference computation (per batch b, time t):
  inputs[o]   = concat(x[b,o,t,:56], forward_probs[b,o,t,:8])          # [8, 64]
  pre_msg[e]  = concat(inputs[recv(e)], inputs[send(e)])               # [56, 128]
  h1          = relu(pre_msg @ W1 + b1)                                # [56, 512]
  msg         = relu(h1 @ W2 + b2)                                     # [56, 512]
  agg[o]      = sum over edges e with recv(e)==o of msg[e]             # [8, 512]
  out[o]      = concat(inputs[o], agg[o]) @ Wn + bn                    # [8, 64]

Sharding: data-parallel over (B=4) x (T-halves=2) -> 8 cores. Each core owns one
(b, t-half) slice: [8 objects, 256 timesteps]. Weights replicated.

Feature-major on-chip layout (features on partitions, (node,time)/(edge,time) on
the free axis), fp16 matmul operands throughout (PE rate is identical to fp32r,
but 2-byte tiles unlock the DVE 2x/4x perf modes and halve SBUF).

L1 is factored through per-node partial products instead of a per-edge matmul:
  A[o] = inputs[o] @ W1[:64]  + b1      (recv half)
  B[o] = inputs[o] @ W1[64:]            (send half)
  h1[e=(r,j)] = relu(A[r] + B[s]),  s = (r+1+j) mod 8
Edges are ordered (recv r, slot j) with the CYCLIC sender convention so that,
with B duplicated along the node axis (16 slots), the whole send-side gather is
a single overlapping access pattern [slot stride 32 in both r and j]. The
edge->node aggregation is a bb-tree over j on 2-byte packed views; the output
head consumes agg via 4 small matmuls plus the input-part matmul.
"""

import contextlib

import numpy as np

import concourse.bacc as bacc
import concourse.mybir as mybir
import concourse.tile as tile
from concourse.bass_types import AP as BassAP
from concourse.bass_utils import run_bass_kernel_spmd
from concourse.masks import make_identity

F32 = mybir.dt.float32
MM_DT = mybir.dt.float16

# Problem constants (hardcoded per the harness contract).
B, O, T = 4, 8, 512
D = 64            # node feature size (56 + 8)
E = 56            # directed edges = O*(O-1)
H = 512           # msg hidden/out size
KK = 64           # K*K output features
TC = 256          # timesteps per core
TB = 32           # timesteps per chunk
NCHUNK = TC // TB
CE = E * TB       # edge columns per chunk (1792)
NN = O * TB       # node columns per chunk (256)
CB = 448          # L2 matmul column block (= 2 recv groups x 7 x TB)
NCB = CE // CB    # 4


def build_nc(mm_dt=MM_DT, repeat=1):
    """Build the per-core Bass program (same program on all 8 cores)."""
    nc = bacc.Bacc("TRN2", target_bir_lowering=False, debug=False)

    xs = nc.dram_tensor("xs", [O, TC, 56], F32, kind="ExternalInput").ap()
    fps = nc.dram_tensor("fps", [O, TC, 8], F32, kind="ExternalInput").ap()
    w1 = nc.dram_tensor("w1", [2 * D, H], F32, kind="ExternalInput").ap()
    b1 = nc.dram_tensor("b1", [H], F32, kind="ExternalInput").ap()
    w2 = nc.dram_tensor("w2", [H, H], F32, kind="ExternalInput").ap()
    b2 = nc.dram_tensor("b2", [H], F32, kind="ExternalInput").ap()
    wn = nc.dram_tensor("wn", [D + H, KK], F32, kind="ExternalInput").ap()
    bn = nc.dram_tensor("bn", [KK], F32, kind="ExternalInput").ap()
    out = nc.dram_tensor("out", [O, TC, KK], F32, kind="ExternalOutput").ap()

    AF = mybir.ActivationFunctionType
    MD = mm_dt

    with tile.TileContext(nc) as tc:
        with (
            tc.tile_pool(name="const", bufs=1) as const,
            tc.tile_pool(name="inp", bufs=3) as inp_pool,
            tc.tile_pool(name="abp", bufs=2) as ab_pool,
            tc.tile_pool(name="h1pre", bufs=2) as h1pre_pool,
            tc.tile_pool(name="h1p", bufs=2) as h1_pool,
            tc.tile_pool(name="msgp", bufs=8) as msg_pool,
            tc.tile_pool(name="tmpp", bufs=3) as tmp_pool,
            tc.tile_pool(name="aggp", bufs=4) as agg_pool,
            tc.tile_pool(name="netp", bufs=2) as net_pool,
            tc.tile_pool(name="orm", bufs=3) as orm_pool,
            tc.tile_pool(name="abps", bufs=4, space="PSUM") as ab_ps,
            tc.tile_pool(name="l2ps", bufs=3, space="PSUM") as l2ps,
            tc.tile_pool(name="opps", bufs=1, space="PSUM") as opps,
        ):
            # ---- constants / weights ----
            ident = const.tile([128, 128], F32)
            make_identity(nc, ident)

            w1s = const.tile([128, H], F32)
            nc.gpsimd.dma_start(w1s[:], w1)
            w2s = const.tile([128, 4 * H], F32)
            for k in range(4):
                nc.gpsimd.dma_start(w2s[:, k * H:(k + 1) * H], w2[k * 128:(k + 1) * 128, :])
            wns = const.tile([128, 5 * KK], F32)
            nc.gpsimd.dma_start(wns[0:64, 4 * KK:5 * KK], wn[0:64, :])
            for k in range(4):
                nc.gpsimd.dma_start(wns[:, k * KK:(k + 1) * KK],
                                    wn[64 + k * 128:64 + (k + 1) * 128, :])
            # fp16 conversions: w1a/w1b on Act (needed first, small); the rest
            # on DVE so the Act queue reaches chunk-0's evicts quickly.
            w1a = const.tile([64, H], MD)          # W1 recv-half rows 0:64
            nc.scalar.copy(w1a[:], w1s[0:64, :])
            w1b = const.tile([64, H], MD)          # W1 send-half rows 64:128
            nc.scalar.copy(w1b[:], w1s[64:128, :])
            w2t = const.tile([128, 4 * H], MD)
            nc.vector.tensor_copy(out=w2t[:], in_=w2s[:])
            wnin = const.tile([64, KK], MD)        # Wn rows 0:64 (node-input part)
            nc.vector.tensor_copy(out=wnin[:], in_=wns[0:64, 4 * KK:5 * KK])
            wnagg = const.tile([128, 4 * KK], MD)  # Wn rows 64+128k
            nc.vector.tensor_copy(out=wnagg[:], in_=wns[:, 0:4 * KK])
            b1t = const.tile([128, 4], F32)
            nc.gpsimd.dma_start(b1t[:], b1.rearrange("(f p) -> p f", p=128))
            b2t = const.tile([128, 4], F32)
            nc.gpsimd.dma_start(b2t[:], b2.rearrange("(f p) -> p f", p=128))
            bnt = const.tile([64, 1], F32)
            nc.gpsimd.dma_start(bnt[:], bn.unsqueeze(1))

            # ---- load node features, transpose to feature-major ----
            inputsT = const.tile([64, O * TC], MD)
            rms = {}
            for th in range(2):
                for o in range(O):
                    rm = inp_pool.tile([128, 64], F32, name=f"rm{th}_{o}", tag="rm")
                    nc.sync.dma_start(rm[:, 0:56], xs[o, th * 128:(th + 1) * 128, :])
                    nc.sync.dma_start(rm[:, 56:64], fps[o, th * 128:(th + 1) * 128, :])
                    rms[(th, o)] = rm

            def make_intr(th, o):
                def unit():
                    rm = rms[(th, o)]
                    tp = opps.tile([64, 128], F32, name="tp", tag="op")
                    nc.tensor.transpose(tp[:], rm[:], ident[:])
                    cb0 = o * TC + th * 128
                    nc.scalar.copy(inputsT[:, cb0:cb0 + 128], tp[:])
                return unit

            for o in range(O):
                make_intr(0, o)()
            # t-half-1 transposes: deferred into chunk-0 stalls for the
            # single-shot build; under For_i they must stay outside the loop.
            input_tq = []
            if repeat > 1:
                for o in range(O):
                    make_intr(1, o)()
            else:
                input_tq = [make_intr(1, o) for o in range(O)]

            inT = inputsT.rearrange("p (o t) -> p o t", o=O)

            def cyc_ap(b2tile, f1):
                """Overlapping send-gather view of the duplicated B tile:
                element (r, j, t) reads node slot 1+r+j (sender (r+1+j)%8)."""
                v = b2tile[:, f1, :]
                return BassAP(v.tensor, v.offset + TB,
                              [list(v.ap[0]), [TB, O], [TB, O - 1], [1, TB]])

            loop_ctx = (tc.For_i(0, repeat, 1,
                                 hint_engines=(mybir.EngineType.PE,))
                        if repeat > 1 else contextlib.nullcontext())
            with loop_ctx:
                pend_l2 = None     # L2 unit thunks from the previous chunk
                pend_agg = None    # agg thunks from the previous chunk
                pend_heads = []    # head thunks (deferred two chunks)

                def make_head(t0, inc, aggT):
                    def head():
                        np_ps = opps.tile([64, NN], F32, name="np_ps", tag="op")
                        nc.tensor.matmul(np_ps[:], wnin[:], inc,
                                         start=True, stop=False)
                        for k in range(4):
                            nc.tensor.matmul(
                                np_ps[:],
                                wnagg[:, k * KK:(k + 1) * KK],
                                aggT[:, k, :],
                                start=False, stop=(k == 3))
                        netoutT = net_pool.tile([64, NN], F32, name="netoutT")
                        nc.scalar.activation(netoutT[:], np_ps[:], AF.Identity,
                                             bias=bnt[:])
                        for hf in range(2):
                            tp2 = opps.tile([128, 64], F32, name="tp2", tag="op")
                            nc.tensor.transpose(
                                tp2[:], netoutT[:, hf * 128:(hf + 1) * 128],
                                ident[0:64, 0:64])
                            outrm = orm_pool.tile([128, 64], F32, name="outrm")
                            nc.vector.tensor_copy(out=outrm[:], in_=tp2[:])
                            o0 = hf * 4
                            nc.sync.dma_start(
                                out[o0:o0 + 4, t0:t0 + TB, :],
                                outrm[:, :])
                    return head

                for c in range(NCHUNK):
                    t0 = c * TB
                    inc = inT[:, :, t0:t0 + TB]          # [64, 8, TB]

                    # ---- A/B node matmuls + evicts to SBUF (2-byte tiles so
                    # the downstream DVE ops run in 2x/4x perf modes) ----
                    A_s = ab_pool.tile([128, 4, NN], MD, name="A_s", tag="ab")
                    B2 = ab_pool.tile([128, 4, 2 * NN], MD, name="B2", tag="ab")
                    ab_ev = []
                    for p in range(2):
                        psA = ab_ps.tile([128, 2, NN], F32, name="psA", tag="abps")
                        psB = ab_ps.tile([128, 2, NN], F32, name="psB", tag="abps")
                        for i in range(2):
                            f1 = 2 * p + i
                            nc.tensor.matmul(
                                psA[:, i, :], w1a[:, f1 * 128:(f1 + 1) * 128],
                                inc, start=True, stop=True)
                        for i in range(2):
                            f1 = 2 * p + i
                            nc.tensor.matmul(
                                psB[:, i, :], w1b[:, f1 * 128:(f1 + 1) * 128],
                                inc, start=True, stop=True)

                        def ev(psA=psA, psB=psB, p=p):
                            for i in range(2):
                                f1 = 2 * p + i
                                nc.scalar.activation(
                                    A_s[:, f1, :], psA[:, i, :], AF.Identity,
                                    bias=b1t[:, f1:f1 + 1])
                                nc.vector.tensor_copy(out=B2[:, f1, 0:NN],
                                                      in_=psB[:, i, :])
                                nc.vector.tensor_copy(out=B2[:, f1, NN:2 * NN],
                                                      in_=B2[:, f1, 0:NN])
                        ab_ev.append(ev)
                    for ev in ab_ev:
                        ev()

                    # ---- h1 build: wide DVE adds (2x) + one 4x relu ----
                    h1 = h1_pool.tile([128, 4, CE], MD, name="h1")
                    h1p = h1pre_pool.tile([128, 4, CE], MD, name="h1p")

                    def make_h1add(p, A_s=A_s, B2=B2, h1p=h1p):
                        def unit():
                            f0 = 2 * p
                            dst = (h1p[:, f0:f0 + 2, :]
                                   .rearrange("p f (r j t) -> p f r j t",
                                              r=O, j=O - 1))
                            a_in = (A_s[:, f0:f0 + 2, :]
                                    .rearrange("p f (r t) -> p f r t", r=O)
                                    .unsqueeze(3)
                                    .broadcast_to([128, 2, O, O - 1, TB]))
                            v = B2[:, f0, :]
                            b_in = BassAP(v.tensor, v.offset + TB,
                                          [list(v.ap[0]), [2 * NN, 2],
                                           [TB, O], [TB, O - 1], [1, TB]])
                            nc.vector.tensor_add(dst, a_in, b_in)
                        return unit

                    def h1relu(h1=h1, h1p=h1p):
                        nc.vector.tensor_relu(out=h1[:], in_=h1p[:])

                    h1q = [make_h1add(0), make_h1add(1), h1relu]

                    # ---- this chunk's L2 / agg units (run next iteration) ----
                    aggT = agg_pool.tile([128, 4, NN], MD, name="aggT")
                    msgs = [msg_pool.tile([128, CE], MD, name=f"msg{f2}", tag="msg")
                            for f2 in range(4)]

                    def make_l2(f2, cb, h1=h1, msgs=msgs):
                        def unit():
                            mp = l2ps.tile([128, CB], F32, name="mp", tag="l2")
                            for k in range(4):
                                nc.tensor.matmul(
                                    mp[:],
                                    w2t[:, k * H + f2 * 128:k * H + (f2 + 1) * 128],
                                    h1[:, k, cb * CB:(cb + 1) * CB],
                                    start=(k == 0), stop=(k == 3))
                            # evict with a permuted write: psum cols are
                            # (rr, j, t); msg layout is (j, r, t) so the agg
                            # tree over j reads 2-byte-packed slices.
                            msgv = (msgs[f2][:]
                                    .rearrange("p (j r t) -> p j r t", j=O - 1, r=O)
                                    [:, :, 2 * cb:2 * cb + 2, :]
                                    .transpose([0, 2, 1, 3]))
                            nc.scalar.activation(
                                msgv,
                                mp[:].rearrange("p (r j t) -> p r j t",
                                                r=2, j=O - 1),
                                AF.Relu, bias=b2t[:, f2:f2 + 1])
                        return unit

                    def make_agg(f2, msgs=msgs, aggT=aggT, c=c):
                        def unit():
                            # DVE runs these at 2x (2-byte packed); Pool takes
                            # two trees since its adds are ~2x slower. The
                            # last chunk's trees stay on DVE (drain latency).
                            eng = (nc.vector if (f2 < 2 or c == NCHUNK - 1)
                                   else nc.gpsimd)
                            mg = msgs[f2][:].rearrange("p (j rt) -> p j rt",
                                                       j=O - 1)
                            tm1 = tmp_pool.tile([128, 3, NN], MD, name="tm1")
                            eng.tensor_add(tm1[:], mg[:, 0:3, :],
                                           mg[:, 3:6, :])
                            tm2 = tmp_pool.tile([128, NN], MD, name="tm2")
                            eng.tensor_add(tm2[:], tm1[:, 0, :],
                                           tm1[:, 1, :])
                            tm3 = tmp_pool.tile([128, NN], MD, name="tm3")
                            eng.tensor_add(tm3[:], tm2[:], tm1[:, 2, :])
                            eng.tensor_add(aggT[:, f2, :], tm3[:],
                                           mg[:, 6, :])
                        return unit

                    # ---- emit: head(c-3), then L2(c-1) interleaved with
                    # h1(c) and agg(c-1) ----
                    if len(pend_heads) >= 3:
                        pend_heads.pop(0)()
                    l2q = list(pend_l2) if pend_l2 else []
                    aggq = list(pend_agg) if pend_agg else []
                    if l2q:
                        for i, u in enumerate(l2q):
                            u()
                            if i % 4 == 1 and h1q:
                                h1q.pop(0)()
                            elif i % 4 == 3 and aggq:
                                aggq.pop(0)()
                            elif input_tq:
                                input_tq.pop(0)()
                    for u in h1q:
                        u()
                        if input_tq:
                            input_tq.pop(0)()
                    for u in aggq:
                        u()

                    # f2 order (2,3,0,1): the Pool-assigned agg trees (f2 2,3)
                    # get their msgs first, hiding Pool's slow adds.
                    F2ORD = (2, 3, 0, 1)
                    pend_l2 = [make_l2(f2, cb) for f2 in F2ORD
                               for cb in range(NCB)]
                    pend_agg = [make_agg(f2) for f2 in F2ORD]
                    pend_heads.append(make_head(t0, inc, aggT))

                # ---- drain the software pipeline: spread the remaining
                # heads into the final L2 block's empty h1 slots ----
                for i, u in enumerate(pend_l2):
                    u()
                    if i % 4 == 3 and pend_agg:
                        pend_agg.pop(0)()
                    elif i % 8 == 1 and len(pend_heads) > 1:
                        pend_heads.pop(0)()
                for u in pend_agg or []:
                    u()
                for hthunk in pend_heads:
                    hthunk()

    nc.compile()
    return nc


_NC_CACHE = {}


def _get_nc():
    key = (MM_DT, 1)
    if key not in _NC_CACHE:
        _NC_CACHE[key] = build_nc(MM_DT, 1)
    return _NC_CACHE[key]


def shard_inputs(x, forward_probs, **_):
    x = np.ascontiguousarray(np.asarray(x, dtype=np.float32))
    fp = np.ascontiguousarray(np.asarray(forward_probs, dtype=np.float32))
    in_maps = []
    for c in range(8):
        b, th = c // 2, c % 2
        in_maps.append({
            "xs": np.ascontiguousarray(x[b, :, th * TC:(th + 1) * TC, :]),
            "fps": np.ascontiguousarray(fp[b, :, th * TC:(th + 1) * TC, :]),
        })
    return in_maps


def kernel(y, x, hidden_states, forward_probs, edge_est, edge_gt,
           W1, b1, W2, b2, Wn, bn, edge2node):
    nc = _get_nc()
    weights = {
        "w1": np.ascontiguousarray(np.asarray(W1, dtype=np.float32)),
        "b1": np.ascontiguousarray(np.asarray(b1, dtype=np.float32)),
        "w2": np.ascontiguousarray(np.asarray(W2, dtype=np.float32)),
        "b2": np.ascontiguousarray(np.asarray(b2, dtype=np.float32)),
        "wn": np.ascontiguousarray(np.asarray(Wn, dtype=np.float32)),
        "bn": np.ascontiguousarray(np.asarray(bn, dtype=np.float32)),
    }
    in_maps = [dict(m, **weights) for m in shard_inputs(x, forward_probs)]
    res = run_bass_kernel_spmd(nc, in_maps, list(range(8)))
    full = np.empty((B, O, T, KK), dtype=np.float32)
    for c in range(8):
        b, th = c // 2, c % 2
        full[b, :, th * TC:(th + 1) * TC, :] = res.results[c]["out"]
    return full.reshape(B, O, T, 8, 8)
